# revision 24
# baseline (speedup 1.0000x reference)
"""Trainium2 Bass kernel for nn_DecoderGRU (attention GRU decoder + vocab head).

Strategy (8 NeuronCores, data-parallel over batch, 8 rows/core):
  - Feature-major layouts ([feature-on-partition, r/batch-on-free]); the GRU
    gates come out feature-major directly, so no transposes anywhere.
  - Hoisted out of the 32-step time loop:
      * feat_proj = features @ attn_W[:E] + attn_b   (fp32r matmul, once)
      * xgx       = emb @ W_ih[:, :E].T + b_ih+b_hh  (fp32r matmul, once)
      * logits    = h_all @ fc_W + fc_b              (fp16 matmul, at end)
  - Per step: h_proj/gh/cgx as bf16 weight-stationary matmuls (FWL); energy
    add + tanh + scores pipelined in two r-halves; softmax without max-sub
    (scores are O(1)); attention weights scattered across partitions by a
    tiny SBUF->SBUF DMA; context as 32 rank-1 PE matmuls contracting r;
    sigmoid via 0.5*(1+tanh(x/2)) so ACT stays on one table set.
"""

import threading

import numpy as np
import ml_dtypes

B, R, E, H, V, L = 64, 49, 512, 512, 10000, 33
T = L - 1            # 32 decode steps
NCORES = 8
BL = B // NCORES     # 8 batch rows per core
KT = E // 128        # 4 k-tiles of 128 for E=H=512
M3H = (3 * H) // 128  # 12 m-tiles for gate dim
RSPLIT = ((0, 25), (25, 49))  # r-halves for the energy pipeline

_BUILD_LOCK = threading.Lock()
_BUILT = {}


def _round_f32r(a):
    """fp32r rounding (drop 13 low mantissa bits, round-to-nearest) on host.

    The BIR verifier requires fp32r-matmul inputs to be produced already
    rounded; for DMA-fed tensors that producer is the host.
    """
    v = np.ascontiguousarray(a, dtype=np.float32).view(np.uint32).astype(np.uint64)
    v = (v + 0x1000) & 0xFFFFE000
    return v.astype(np.uint32).view(np.float32)


def _build(has_fcb=True):
    import concourse.mybir as mybir
    import concourse.tile as tile
    from concourse import bacc

    F32 = mybir.dt.float32
    F32R = mybir.dt.float32r
    BF16 = mybir.dt.bfloat16
    F16 = mybir.dt.float16
    AF = mybir.ActivationFunctionType
    OP = mybir.AluOpType

    nc = bacc.Bacc("TRN2", target_bir_lowering=False, debug=False,
                   num_devices=NCORES)

    # ---- DRAM I/O ----
    featsT_d = nc.dram_tensor("featsT", [E, R, BL], F32R, kind="ExternalInput")
    featsb_d = nc.dram_tensor("featsb", [E, BL, R], BF16, kind="ExternalInput")
    embT_d = nc.dram_tensor("embT", [E, T * BL], F32R, kind="ExternalInput")
    attn_We_d = nc.dram_tensor("attn_We", [E, H], F32R, kind="ExternalInput")
    attn_Wh_d = nc.dram_tensor("attn_Wh", [H, H], BF16, kind="ExternalInput")
    W_hhT_d = nc.dram_tensor("W_hhT", [H, 3 * H], BF16, kind="ExternalInput")
    W_ihcT_d = nc.dram_tensor("W_ihcT", [E, 3 * H], BF16, kind="ExternalInput")
    W_iheT_d = nc.dram_tensor("W_iheT", [E, 3 * H], F32R, kind="ExternalInput")
    vw_d = nc.dram_tensor("vw", [H, 1], BF16, kind="ExternalInput")
    bsum_d = nc.dram_tensor("bsum", [3 * H, 1], F32, kind="ExternalInput")
    attnb_d = nc.dram_tensor("attnb", [H, 1], F32, kind="ExternalInput")
    fcW_d = nc.dram_tensor("fcW", [H, V], F16, kind="ExternalInput")
    out_d = nc.dram_tensor("out", [T * BL, V], F32, kind="ExternalOutput")

    r3 = lambda ap: ap.rearrange("(kt p) m -> p kt m", p=128)

    with tile.TileContext(nc) as tc:
        with tc.tile_pool(name="persist", bufs=1) as P1:
            # ---- resident tensors (recurrence weights on the Pool queue) ----
            feats_bf = P1.tile([128, KT, BL, R], BF16)
            nc.gpsimd.dma_start(feats_bf[:], featsb_d.ap().rearrange(
                "(kt p) b r -> p kt b r", p=128))
            attn_Wh = P1.tile([128, KT, H], BF16)
            nc.gpsimd.dma_start(attn_Wh[:], r3(attn_Wh_d.ap()))
            W_hhT = P1.tile([128, KT, 3 * H], BF16)
            nc.gpsimd.dma_start(W_hhT[:], r3(W_hhT_d.ap()))
            W_ihcT = P1.tile([128, KT, 3 * H], BF16)
            nc.gpsimd.dma_start(W_ihcT[:], r3(W_ihcT_d.ap()))
            vw = P1.tile([128, KT, 1], BF16)
            nc.gpsimd.dma_start(vw[:], r3(vw_d.ap()))
            bsum = P1.tile([128, M3H, 1], F32)
            nc.gpsimd.dma_start(bsum[:], r3(bsum_d.ap()))
            attnb = P1.tile([128, KT, 1], F32)
            nc.gpsimd.dma_start(attnb[:], r3(attnb_d.ap()))
            ones_row = P1.tile([1, 128], F32)
            nc.vector.memset(ones_row[:], 1.0)
            ones_b = P1.tile([1, 128], BF16)
            nc.vector.memset(ones_b[:], 1.0)
            # fc weights tile (DMA issued after precompute, below)
            fcW = P1.tile([128, KT, V], F16)
            # fp16 hidden-state history (columns t*BL+b), filled per step
            h_all = P1.tile([128, KT, T * BL], F16)
            # feat_proj (tanh-input bias from features), filled below
            fpT = P1.tile([128, KT, R, BL], BF16)
            # xgx: embedding side of gate preactivations + biases
            xgxT = P1.tile([128, M3H, T * BL], F32)

            with tc.tile_pool(name="pre", bufs=1) as PP, \
                 tc.tile_pool(name="pre_ps", bufs=2, space="PSUM") as PPS:
                # feat_proj = features @ attn_W[:E] + attn_b  (feature-major)
                featsT = PP.tile([128, KT, R, BL], F32R)
                nc.sync.dma_start(featsT[:], featsT_d.ap().rearrange(
                    "(kt p) r b -> p kt r b", p=128))
                attn_We = PP.tile([128, KT, H], F32R)
                nc.sync.dma_start(attn_We[:], r3(attn_We_d.ap()))
                for mo in range(KT):
                    ps = PPS.tile([128, R * BL], F32, name="fp_ps")
                    for kt in range(KT):
                        nc.tensor.matmul(
                            ps[:], attn_We[:, kt, mo * 128:(mo + 1) * 128],
                            featsT[:, kt].rearrange("p r b -> p (r b)"),
                            start=(kt == 0), stop=(kt == KT - 1))
                    nc.vector.tensor_scalar(
                        out=fpT[:, mo].rearrange("p r b -> p (r b)"),
                        in0=ps[:], scalar1=attnb[:, mo], scalar2=None,
                        op0=OP.add)
                # xgx = emb @ W_ih[:, :E].T + (b_ih + b_hh)
                W_iheT = PP.tile([128, KT, 3 * H], F32R)
                nc.scalar.dma_start(W_iheT[:], r3(W_iheT_d.ap()))
                embT = PP.tile([128, KT, T * BL], F32R)
                nc.scalar.dma_start(embT[:], r3(embT_d.ap()))
                for m in range(M3H):
                    ps = PPS.tile([128, T * BL], F32, name="xg_ps")
                    for kt in range(KT):
                        nc.tensor.matmul(
                            ps[:], W_iheT[:, kt, m * 128:(m + 1) * 128],
                            embT[:, kt], start=(kt == 0), stop=(kt == KT - 1))
                    nc.vector.tensor_scalar(
                        out=xgxT[:, m], in0=ps[:], scalar1=bsum[:, m],
                        scalar2=None, op0=OP.add)

            # fc weight prefetch: issued after the precompute's input DMAs so
            # those go first in the queue; finishes during the recurrence
            for kt in range(KT):
                nc.sync.dma_start(fcW[:, kt], r3(fcW_d.ap())[:, kt])

            # ---- recurrence ----
            with tc.tile_pool(name="state", bufs=2) as PST, \
                 tc.tile_pool(name="scratch", bufs=2) as PSC, \
                 tc.tile_pool(name="gates", bufs=2) as PG, \
                 tc.tile_pool(name="ps_hp", bufs=2, space="PSUM") as PS_HP, \
                 tc.tile_pool(name="ps_sc", bufs=2, space="PSUM") as PS_SC, \
                 tc.tile_pool(name="ps_ctx", bufs=2, space="PSUM") as PS_CTX, \
                 tc.tile_pool(name="ps_g", bufs=1, space="PSUM") as PS_G:
                h_T = PST.tile([128, KT, BL], BF16, name="h_init")
                nc.vector.memset(h_T[:], 0.0)

                for t in range(T):
                    # gh = W_hh @ h (fills PE while attention runs)
                    g_gh = PS_G.tile([128, M3H, BL], F32, name="g_gh")
                    g_cgx = PS_G.tile([128, M3H, BL], F32, name="g_cgx")
                    for m in range(M3H):
                        for kt in range(KT):
                            nc.tensor.matmul(
                                g_gh[:, m], W_hhT[:, kt, m * 128:(m + 1) * 128],
                                h_T[:, kt], start=(kt == 0),
                                stop=(kt == KT - 1))

                    xg = xgxT[:, :, t * BL:(t + 1) * BL]

                    # h_proj = attn_W[E:] @ h   (feature-major out)
                    hp = PS_HP.tile([128, KT, BL], F32, name="hp")
                    for mo in range(KT):
                        for kt in range(KT):
                            nc.tensor.matmul(
                                hp[:, mo], attn_Wh[:, kt, mo * 128:(mo + 1) * 128],
                                h_T[:, kt], start=(kt == 0), stop=(kt == KT - 1))

                    # energy = tanh(feat_proj + h_proj); scores = v_w . energy
                    # pipelined in two r-halves across DVE -> ACT -> PE
                    hp_bf = PSC.tile([128, KT, BL], BF16, name="hp_bf")
                    nc.vector.tensor_copy(hp_bf[:], hp[:])
                    sc = PS_SC.tile([1, R, BL], F32, name="sc", bufs=1)
                    en_b = PSC.tile([128, KT, R, BL], BF16, name="en_b", bufs=1)
                    for (r0, r1) in RSPLIT:
                        nr = r1 - r0
                        en_f = PSC.tile([128, KT, 25, BL], BF16,
                                        name=f"en_f{r0}", bufs=1)
                        nc.vector.tensor_tensor(
                            out=en_f[:, :, :nr], in0=fpT[:, :, r0:r1],
                            in1=hp_bf[:, :, None, :].to_broadcast(
                                (128, KT, nr, BL)),
                            op=OP.add)
                        nc.scalar.activation(
                            en_b[:, :, r0:r1], en_f[:, :, :nr], AF.Tanh)
                        for kt in range(KT):
                            nc.tensor.matmul(
                                sc[:, r0:r1].rearrange("p r b -> p (r b)"),
                                vw[:, kt],
                                en_b[:, kt, r0:r1].rearrange("p r b -> p (r b)"),
                                start=(kt == 0), stop=(kt == KT - 1))

                    # gate pre-add needing only gh + constants: emitted
                    # after the energy chain so DVE prioritizes the chain
                    rzpre = PG.tile([128, 8, BL], F32, name="rzpre")
                    nc.vector.tensor_tensor(
                        out=rzpre[:], in0=g_gh[:, 0:8], in1=xg[:, 0:8],
                        op=OP.add)

                    # softmax, unnormalized (scores are O(1): no max-sub;
                    # the 1/sum lands on the context below). bf16 exp is
                    # replicated across partitions by a PE rank-1 broadcast;
                    # the per-b 1/sum the same way, overlapping each other.
                    ex = PSC.tile([1, BL, R], BF16, name="ex")
                    nc.scalar.activation(
                        ex[:].rearrange("p b r -> p r b"), sc[:], AF.Exp)
                    exb_ps = PS_CTX.tile([128, BL * R], F32, name="exb_ps")
                    nc.tensor.matmul(
                        exb_ps[:], ones_b[:], ex[:].rearrange("p b r -> p (b r)"),
                        start=True, stop=True)
                    exb = PSC.tile([128, BL, R], BF16, name="exb", bufs=1)
                    nc.vector.tensor_copy(
                        exb[:].rearrange("p b r -> p (b r)"), exb_ps[:])
                    # context = sum_r attn * feats (bf16 DVE mult + reduce)
                    prod = PSC.tile([128, KT, BL, R], BF16, name="prod",
                                    bufs=1)
                    nc.vector.tensor_tensor(
                        out=prod[:], in0=feats_bf[:],
                        in1=exb[:, None].to_broadcast((128, KT, BL, R)),
                        op=OP.mult)
                    ctx_u = PSC.tile([128, KT, BL], F32, name="ctx_u")
                    nc.vector.tensor_reduce(
                        out=ctx_u[:], in_=prod[:],
                        axis=mybir.AxisListType.X, op=OP.add)
                    s_sum = PSC.tile([1, BL], F32, name="s_sum")
                    nc.vector.tensor_reduce(
                        out=s_sum[:], in_=ex[:],
                        axis=mybir.AxisListType.X, op=OP.add)
                    rec = PSC.tile([1, BL], F32, name="rec")
                    nc.vector.reciprocal(rec[:], s_sum[:])
                    recb_ps = PS_CTX.tile([128, BL], F32, name="recb_ps",
                                          bufs=1)
                    nc.tensor.matmul(recb_ps[:], ones_row[:], rec[:],
                                     start=True, stop=True)
                    recb = PSC.tile([128, BL], F32, name="recb")
                    nc.vector.tensor_copy(recb[:], recb_ps[:])
                    ctx_bf = PSC.tile([128, KT, BL], BF16, name="ctx_bf")
                    nc.vector.tensor_tensor(
                        out=ctx_bf[:], in0=ctx_u[:],
                        in1=recb[:, None, :].to_broadcast((128, KT, BL)),
                        op=OP.mult)

                    # cgx = W_ih[:, E:] @ context
                    for m in range(M3H):
                        for kt in range(KT):
                            nc.tensor.matmul(
                                g_cgx[:, m], W_ihcT[:, kt, m * 128:(m + 1) * 128],
                                ctx_bf[:, kt], start=(kt == 0),
                                stop=(kt == KT - 1))

                    # gates: r,z = 0.5*(1+tanh(0.5*x)); n = tanh(xn + r*hn)
                    xn_tot = PG.tile([128, 4, BL], F32, name="xn_tot")
                    nc.vector.tensor_tensor(
                        out=xn_tot[:], in0=g_cgx[:, 8:12], in1=xg[:, 8:12],
                        op=OP.add)
                    rz_t = PG.tile([128, 8, BL], F32, name="rz_t")
                    nc.vector.tensor_tensor(
                        out=rz_t[:], in0=g_cgx[:, 0:8], in1=rzpre[:],
                        op=OP.add)
                    rz_h = PG.tile([128, 8, BL], F32, name="rz_h")
                    nc.scalar.activation(rz_h[:], rz_t[:], AF.Tanh, scale=0.5)
                    rz = PG.tile([128, 8, BL], F32, name="rz")
                    nc.vector.tensor_scalar(
                        out=rz[:], in0=rz_h[:], scalar1=0.5, scalar2=0.5,
                        op0=OP.mult, op1=OP.add)
                    n_p = PG.tile([128, 4, BL], F32, name="n_p")
                    nc.vector.tensor_tensor(
                        out=n_p[:], in0=rz[:, 0:4], in1=g_gh[:, 8:12],
                        op=OP.mult)
                    nc.vector.tensor_tensor(
                        out=n_p[:], in0=n_p[:], in1=xn_tot[:], op=OP.add)
                    n_t = PG.tile([128, 4, BL], F32, name="n_t")
                    nc.scalar.activation(n_t[:], n_p[:], AF.Tanh)
                    # h_new = n + z*(h - n)
                    hmn = PG.tile([128, 4, BL], F32, name="hmn")
                    nc.vector.tensor_tensor(
                        out=hmn[:], in0=h_T[:], in1=n_t[:], op=OP.subtract)
                    h_new = PST.tile([128, KT, BL], BF16, name="h_new")
                    nc.vector.tensor_tensor(
                        out=hmn[:], in0=rz[:, 4:8], in1=hmn[:], op=OP.mult)
                    nc.vector.tensor_tensor(
                        out=h_new[:], in0=n_t[:], in1=hmn[:], op=OP.add)
                    # fp16 history for the fc matmul
                    nc.vector.tensor_copy(
                        h_all[:, :, t * BL:(t + 1) * BL], h_new[:])
                    h_T = h_new

            # ---- final fc: logits = h_all.T @ fc_W (+ fc_b) ----
            NCH = (V + 511) // 512  # 20 chunks, last = 272
            with tc.tile_pool(name="fc_ps", bufs=4, space="PSUM") as FPS, \
                 tc.tile_pool(name="fc_sb", bufs=4) as FSB, \
                 tc.tile_pool(name="fc_bias", bufs=1) as FB:
                fcb = None
                if has_fcb:
                    fcb_d = nc.dram_tensor("fcb", [1, V], F16,
                                           kind="ExternalInput")
                    fcb = FB.tile([128, V], F16)
                    nc.sync.dma_start(fcb[:], fcb_d.ap().to_broadcast((128, V)))
                for mo in range(2):
                    rows = slice(mo * 128, (mo + 1) * 128)
                    for ch in range(NCH):
                        nv = min(512, V - ch * 512)
                        cols = slice(ch * 512, ch * 512 + nv)
                        ps = FPS.tile([128, 512], F32, name="fc_ps")
                        for kt in range(KT):
                            nc.tensor.matmul(
                                ps[:, :nv], h_all[:, kt, rows],
                                fcW[:, kt, cols], start=(kt == 0),
                                stop=(kt == KT - 1))
                        ot = FSB.tile([128, 512], F32, name="fc_ot")
                        k = (mo * NCH + ch) % 3
                        if has_fcb:
                            nc.vector.tensor_tensor(
                                out=ot[:, :nv], in0=ps[:, :nv],
                                in1=fcb[:, cols], op=OP.add)
                        elif k == 2:
                            nc.scalar.copy(ot[:, :nv], ps[:, :nv])
                        else:
                            nc.vector.tensor_copy(ot[:, :nv], ps[:, :nv])
                        dma_eng = nc.sync if (mo * NCH + ch) % 2 == 0 else nc.scalar
                        dma_eng.dma_start(out_d.ap()[rows, cols], ot[:, :nv])

    nc.compile()
    return nc


def _get_built(has_fcb=True):
    with _BUILD_LOCK:
        if has_fcb not in _BUILT:
            _BUILT[has_fcb] = _build(has_fcb)
    return _BUILT[has_fcb]


def kernel(features, captions, embed_table, attn_W, attn_b, v_w,
           W_ih, W_hh, b_ih, b_hh, fc_W, fc_b):
    from concourse.bass_utils import run_bass_kernel_spmd

    features = np.asarray(features, dtype=np.float32)
    captions = np.asarray(captions)
    embed_table = np.asarray(embed_table, dtype=np.float32)
    attn_W = np.asarray(attn_W, dtype=np.float32)
    attn_b = np.asarray(attn_b, dtype=np.float32)
    v_w = np.asarray(v_w, dtype=np.float32)
    W_ih = np.asarray(W_ih, dtype=np.float32)
    W_hh = np.asarray(W_hh, dtype=np.float32)
    b_ih = np.asarray(b_ih, dtype=np.float32)
    b_hh = np.asarray(b_hh, dtype=np.float32)
    fc_W = np.asarray(fc_W, dtype=np.float32)
    fc_b = np.asarray(fc_b, dtype=np.float32)

    has_fcb = bool(np.any(fc_b))
    nc = _get_built(has_fcb)

    bf16 = ml_dtypes.bfloat16
    shared = {
        "attn_We": _round_f32r(attn_W[:E]),
        "attn_Wh": attn_W[E:].astype(bf16),
        "W_hhT": np.ascontiguousarray(W_hh.T).astype(bf16),
        "W_ihcT": np.ascontiguousarray(W_ih[:, E:].T).astype(bf16),
        "W_iheT": _round_f32r(W_ih[:, :E].T),
        "vw": v_w[:, None].astype(bf16),
        "bsum": np.ascontiguousarray((b_ih + b_hh)[:, None]),
        "attnb": np.ascontiguousarray(attn_b[:, None]),
        "fcW": fc_W.astype(np.float16),
    }
    if has_fcb:
        shared["fcb"] = fc_b[None, :].astype(np.float16)
    emb = embed_table[captions[:, :T].astype(np.int64)]  # [B, T, E]
    in_maps = []
    for c in range(NCORES):
        rows = slice(c * BL, (c + 1) * BL)
        m = dict(shared)
        m["featsT"] = _round_f32r(features[rows].transpose(2, 1, 0))
        m["featsb"] = features[rows].transpose(2, 0, 1).astype(bf16)
        m["embT"] = _round_f32r(
            emb[rows].transpose(2, 1, 0).reshape(E, T * BL))
        in_maps.append(m)

    res = run_bass_kernel_spmd(nc, in_maps, core_ids=list(range(NCORES)))

    out = np.empty((B, T, V), dtype=np.float32)
    for c in range(NCORES):
        # rows of per-core output are t*BL + b_local
        out[c * BL:(c + 1) * BL] = (
            res.results[c]["out"].reshape(T, BL, V).transpose(1, 0, 2))
    return out
